# revision 12
# baseline (speedup 1.0000x reference)
"""Trainium2 Bass kernel for nn_ARIMA_59373627900094 (flow-sampling ARIMA MLP).

Math: 100 Euler steps of x <- x + dt*(MLP([x,t]) - noise), x0 = noise,
over B*C = 16384 independent rows of dim 97 (MLP: 98 -> 100 -> 100 -> 97, tanh).

Device formulation (per row, feature-major) avoids materializing x entirely:
  z~_i = W1x @ x_i - i*v   kept in PSUM (accumulating matmuls)
  h1 = tanh(z~_i + btab[:,i])            # btab folds b1, the time input, and i*v
  h2 = tanh(W2 @ h1 + b2)
  z~_{i+1} = z~_i + G @ h2 - dt*(W1x @ noise)   # two accumulating matmuls
  S += h2                                 # VectorE
  out = dt*W3 @ S + b3                    # exact: 100*dt*noise cancels x0

Sharding: pure data parallel, batch dim across 8 cores (2048 rows each).
"""

import sys

for _p in ("/opt/trn_rl_repo",):
    if _p not in sys.path:
        sys.path.insert(0, _p)

import numpy as np

B, Q, C, H, NSTEPS = 1024, 96, 16, 100, 100
NCORES = 8
FEAT = Q + 1          # 97
ROWS = B * C          # 16384
RPC = ROWS // NCORES  # 2048 rows per core
NCHUNK = 2
CHUNK = RPC // NCHUNK  # 1024
MMF = 512             # matmul free dim (one PSUM bank of fp32)

_COMPILED = {}


def _build():
    import concourse.bacc as bacc
    import concourse.bass as bass
    import concourse.tile as tile

    mybir = bass.mybir
    f32 = mybir.dt.float32
    f32r = mybir.dt.float32r
    Tanh = mybir.ActivationFunctionType.Tanh
    add = mybir.AluOpType.add

    nc = bacc.Bacc("TRN2", target_bir_lowering=False, debug=False,
                   num_devices=NCORES)

    bf16 = mybir.dt.bfloat16
    noise_ext = nc.declare_dram_parameter("noise", [FEAT, RPC], bf16, isOutput=False)
    w1xT_ext = nc.declare_dram_parameter("w1xT", [FEAT, 128], bf16, isOutput=False)
    w2T_ext = nc.declare_dram_parameter("w2T", [H, 128], bf16, isOutput=False)
    gT_ext = nc.declare_dram_parameter("gT", [H, 128], bf16, isOutput=False)
    cT_ext = nc.declare_dram_parameter("cT", [FEAT, 128], bf16, isOutput=False)
    w3dT_ext = nc.declare_dram_parameter("w3dT", [H, 128], f32r, isOutput=False)
    btab_ext = nc.declare_dram_parameter("btab", [H, 128], f32, isOutput=False)
    out_ext = nc.declare_dram_parameter("out", [FEAT, RPC], f32, isOutput=True)

    with tile.TileContext(nc) as tc:
        with tc.tile_pool(name="const", bufs=1) as cp, \
             tc.tile_pool(name="work", bufs=3) as wp, \
             tc.tile_pool(name="zp", bufs=1, space="PSUM") as zp, \
             tc.tile_pool(name="mp", bufs=1, space="PSUM") as mp:

            n_sb = cp.tile([FEAT, RPC], bf16, tag="n")
            w1xT = cp.tile([FEAT, 128], bf16, tag="w1xT")
            w2T = cp.tile([H, 128], bf16, tag="w2T")
            gT = cp.tile([H, 128], bf16, tag="gT")
            cT = cp.tile([FEAT, 128], bf16, tag="cT")
            w3dT = cp.tile([H, 128], f32r, tag="w3dT")
            btab = cp.tile([H, 128], f32, tag="btab")
            S = cp.tile([H, RPC], f32r, tag="S")

            nc.sync.dma_start(out=n_sb[:, 0:CHUNK], in_=noise_ext[:, 0:CHUNK])
            nc.sync.dma_start(out=w1xT[:], in_=w1xT_ext[:])
            nc.sync.dma_start(out=btab[:], in_=btab_ext[:])
            nc.gpsimd.dma_start(out=n_sb[:, CHUNK:RPC],
                                in_=noise_ext[:, CHUNK:RPC])
            nc.gpsimd.dma_start(out=w2T[:], in_=w2T_ext[:])
            nc.scalar.dma_start(out=gT[:], in_=gT_ext[:])
            nc.scalar.dma_start(out=cT[:], in_=cT_ext[:])
            nc.scalar.dma_start(out=w3dT[:], in_=w3dT_ext[:])
            scratch = nc.dram_tensor("scratch", [1, 128], f32r)

            # z~_0 = W1x @ noise, one persistent 2-bank PSUM tile per chunk
            z = []
            for ch in range(NCHUNK):
                zt = zp.tile([H, CHUNK], f32, tag=f"z{ch}")
                z.append(zt)
                for s in range(CHUNK // MMF):
                    col = ch * CHUNK + s * MMF
                    nc.tensor.matmul(
                        zt[:, s * MMF:(s + 1) * MMF],
                        lhsT=w1xT[:, :H],
                        rhs=n_sb[:, col:col + MMF],
                        start=True, stop=False)

            for i in range(NSTEPS):
                for ch in range(NCHUNK):
                    c0 = ch * CHUNK
                    h1 = wp.tile([H, CHUNK], bf16, tag=f"h1_{ch}")
                    nc.scalar.activation(h1[:], z[ch][:], Tanh,
                                         bias=btab[:, i:i + 1], scale=1.0)
                    ps2 = mp.tile([H, CHUNK], f32, tag=f"ps2_{ch}")
                    for s in range(CHUNK // MMF):
                        sl = slice(s * MMF, (s + 1) * MMF)
                        nc.tensor.matmul(ps2[:, sl], lhsT=w2T[:, :H],
                                         rhs=h1[:, sl], start=True, stop=True)
                    if i < NSTEPS - 1:
                        for s in range(CHUNK // MMF):
                            sl = slice(s * MMF, (s + 1) * MMF)
                            col = c0 + s * MMF
                            nc.tensor.matmul(z[ch][:, sl], lhsT=cT[:, :H],
                                             rhs=n_sb[:, col:col + MMF],
                                             start=False, stop=False)
                    h2 = wp.tile([H, CHUNK], bf16, tag=f"h2_{ch}")
                    nc.scalar.activation(h2[:], ps2[:], Tanh,
                                         bias=btab[:, NSTEPS:NSTEPS + 1], scale=1.0)
                    if i == NSTEPS - 8 and ch == 0:
                        nc.sync.dma_start(out=scratch[0:1, 0:64],
                                          in_=S[0:1, 0:64])
                        nc.scalar.dma_start(out=scratch[0:1, 64:128],
                                            in_=S[1:2, 0:64])
                    if i == 0:
                        nc.vector.tensor_copy(S[:, c0:c0 + CHUNK], h2[:])
                    else:
                        nc.vector.tensor_tensor(S[:, c0:c0 + CHUNK],
                                                S[:, c0:c0 + CHUNK], h2[:], add)
                    if i < NSTEPS - 1:
                        for s in range(CHUNK // MMF):
                            sl = slice(s * MMF, (s + 1) * MMF)
                            nc.tensor.matmul(z[ch][:, sl], lhsT=gT[:, :H],
                                             rhs=h2[:, sl],
                                             start=False, stop=(i == NSTEPS - 2))

            # out = dt*W3 @ S + b3
            for ch in range(NCHUNK):
                c0 = ch * CHUNK
                pO = mp.tile([FEAT, CHUNK], f32, tag=f"ps2_{ch}")
                for s in range(CHUNK // MMF):
                    sl = slice(s * MMF, (s + 1) * MMF)
                    nc.tensor.matmul(pO[:, sl], lhsT=w3dT[:, :FEAT],
                                     rhs=S[:, c0 + s * MMF:c0 + (s + 1) * MMF],
                                     start=True, stop=True)
                o_sb = wp.tile([FEAT, CHUNK], f32, tag=f"o_{ch}")
                nc.vector.tensor_scalar_add(o_sb[:], pO[:], btab[:FEAT, NSTEPS + 1:NSTEPS + 2])
                (nc.sync if ch == 0 else nc.scalar).dma_start(out=out_ext[:, c0:c0 + CHUNK], in_=o_sb[:])

    nc.compile()
    return nc


def _get_nc():
    if "nc" not in _COMPILED:
        _COMPILED["nc"] = _build()
    return _COMPILED["nc"]


def _host_prep(series, rand_error, W1, b1, W2, b2, W3, b3):
    dt = np.float32(1.0 / NSTEPS)
    noise = np.concatenate([series, rand_error], axis=1)        # (B, 97, C)
    n = np.ascontiguousarray(
        noise.transpose(1, 0, 2).reshape(FEAT, ROWS), np.float32)  # (97, rows)

    W1x = W1[:, :FEAT]                                          # (100, 97)
    w1t = W1[:, FEAT]                                           # (100,)
    v = dt * (W1x @ b3)                                         # (100,)
    steps = np.arange(NSTEPS, dtype=np.float32)
    btab = (b1[:, None] + np.outer(w1t, steps / NSTEPS)
            + np.outer(v, steps)).astype(np.float32)            # (100, 100)
    b3p = np.zeros(H, np.float32)
    b3p[:FEAT] = b3
    btab = np.concatenate([btab, b2[:, None], b3p[:, None]], axis=1)  # (100, 102)
    btab = np.concatenate([btab, np.zeros((H, 26), np.float32)], axis=1)  # (100, 128)

    import ml_dtypes
    bf16 = ml_dtypes.bfloat16

    def pad128(a):
        out = np.zeros((a.shape[0], 128), a.dtype)
        out[:, :a.shape[1]] = a
        return out

    shared = {
        "w1xT": pad128(np.ascontiguousarray(W1x.T.astype(bf16))),
        "w2T": pad128(np.ascontiguousarray(W2.T.astype(bf16))),
        "gT": pad128(np.ascontiguousarray((dt * (W1x @ W3)).T.astype(bf16))),
        "cT": pad128(np.ascontiguousarray((-dt * W1x).T.astype(bf16))),
        "w3dT": pad128(np.ascontiguousarray((dt * W3).T, np.float32)),
        "btab": np.ascontiguousarray(btab, np.float32),
    }
    in_maps = []
    for core in range(NCORES):
        m = dict(shared)
        m["noise"] = np.ascontiguousarray(n[:, core * RPC:(core + 1) * RPC].astype(bf16))
        in_maps.append(m)
    return in_maps


def kernel(series, rand_error, W1, b1, W2, b2, W3, b3, _trace=False,
           _tmpdir=None, _nc_out=None):
    from concourse.bass_utils import run_bass_kernel_spmd

    args = [np.asarray(a, np.float32) for a in
            (series, rand_error, W1, b1, W2, b2, W3, b3)]
    in_maps = _host_prep(*args)
    nc = _get_nc()
    if _nc_out is not None:
        _nc_out.append(nc)
    res = run_bass_kernel_spmd(nc, in_maps, core_ids=list(range(NCORES)),
                               trace=_trace, tmpdir=_tmpdir)
    outs = [np.asarray(res.results[i]["out"]) for i in range(NCORES)]
    full = np.concatenate(outs, axis=1)                         # (97, rows)
    out = full.reshape(FEAT, B, C).transpose(1, 0, 2)           # (B, 97, C)
    if _trace:
        return np.ascontiguousarray(out), res
    return np.ascontiguousarray(out)
